# revision 6
# baseline (speedup 1.0000x reference)
"""GCN (2-layer) on 8 Trainium2 NeuronCores — v2.

Strategy (node-range sharding per the hint):
- Nodes sharded by range (25k/core); each edge lives on the core owning its
  DESTINATION node; tiny weights replicated.
- All per-edge routing is done host-side between device programs, as pure
  byte movement of device-produced values. Every FP operation on values
  happens on device.
- Scatter layout: destination nodes bucketed by padded in-degree class
  L = 4*ceil((deg+1)/4) (slot 0 = self-loop); within a class, nodes sit at
  (partition p, block b) of a [128, L, F, nbj] grid with the slot dim l
  OUTERMOST, so the segment-sum fold tree operates on large fully
  contiguous slabs (keeps the DVE 2x fast path; inner-dim slices would
  pay a per-row penalty).
- Normalization is pre-folded on device in PA: ew' = ew * dis[col] (self
  slot gets 1.0*dis), x' = x * dis. Then layer aggregation is simply
  v = sum_slots ew'_slot * val_slot with NO per-node epilogue.
- Layer-1 MLP runs on the Tensor engine: vd = [v,1]*dis flattened to a
  [4, CB] moving tensor; z = W1e^T @ vd accumulated into 32-row PSUM
  stripes (16 real rows + 16 zeros; 4 stripes fill a [128,512] bank;
  PE tile_position requires 32-aligned output bases); relu on Act engine;
  layer-2 ys2 = (block-diag W2)^T @ z per bank. dis enters via vd so
  relu(dis*z) = dis*relu(z) and ys2 = (dis*h) @ W2 directly.

Three SPMD programs: PA (deg/dis/prescale) -> PB (L1 aggregate + MLP)
-> PC (L2 aggregate + bias).
"""
import sys

sys.path.insert(0, "/opt/trn_rl_repo")

import numpy as np
from ml_dtypes import bfloat16

import bass_rust
from concourse import bass, mybir
from concourse.bass_utils import run_bass_kernel_spmd
import concourse.tile as tile

import os as _os

PROGRAM_TIMES_NS = []   # (name, exec_time_ns) per device program of last kernel() call
_LAST_ST = [None]       # structure of last kernel() call (for tests)


def _enable_tracing():
    import types
    import antenv
    if 'antenv.axon_hooks' in sys.modules:
        return True
    try:
        from trn_agent_boot.trn_boot import _ntff_profile_via_ctypes
        hook = _ntff_profile_via_ctypes('/opt/axon/libaxon_pjrt.so')
    except Exception:
        return False
    mod = types.ModuleType('antenv.axon_hooks')
    mod.get_axon_ntff_profile_hook = lambda: hook
    mod.set_axon_ntff_profile_hook = lambda h: None
    sys.modules['antenv.axon_hooks'] = mod
    antenv.axon_hooks = mod
    import concourse.bass_utils as _bu
    _bu.upload_artifacts = lambda tmpdir: f"local://{tmpdir}"
    return True


def _run(nc, in_maps, name):
    trace = bool(_os.environ.get('GCN_TRACE')) and _enable_tracing()
    r = run_bass_kernel_spmd(nc, in_maps, core_ids=CORE_IDS, trace=trace)
    if trace:
        PROGRAM_TIMES_NS.append((name, r.exec_time_ns))
    return r.results

S = 8
N = 200000
NS = N // S
P = 128
CORE_IDS = list(range(S))
FP = mybir.dt.float32
BF = mybir.dt.bfloat16
MUL = mybir.AluOpType.mult
ADD = mybir.AluOpType.add

def _ceil(a, b):
    return -(-a // b)


# --------------------------------------------------------------------------
# walrus on this toolchain accepts at most ONE sync-wait per instruction;
# Tile emits several at DAG joins / kernel-tail drain. Hoist excess waits
# onto fresh same-engine NoOps inserted immediately before the violator.
def legalize_waits(nc):
    nop_idx = 0
    for f in nc.m.functions:
        for bb in f.blocks:
            il = bb.instructions
            if not any(
                inst.sync_info is not None
                and len(inst.sync_info.on_wait or []) > 1
                for inst in il
            ):
                continue
            new_il = []
            for inst in il:
                si = inst.sync_info
                w = list(si.on_wait or []) if si is not None else []
                if len(w) > 1:
                    for extra in w[:-1]:
                        nop = mybir.InstNoOp(
                            name=f"I-waitsplit-{nop_idx}", ins=[], outs=[]
                        )
                        nop_idx += 1
                        nop.engine = inst.engine
                        nop.sync_info = bass_rust.SyncInfo(
                            on_wait=[extra], on_update=[]
                        )
                        new_il.append(nop)
                    si.on_wait = [w[-1]]
                new_il.append(inst)
            bb.instructions = new_il


# --------------------------------------------------------------------------
# host-side structure building (integer routing only — no FP math on values)
class _O:
    pass


def build_structs(row, col, ew):
    row = np.asarray(row, np.int64)
    col = np.asarray(col, np.int64)
    st = _O()
    cores = []
    for c in range(S):
        cs = _O()
        m = (col // NS) == c
        cs.erow = row[m]                       # global source node
        cs.ecol = (col[m] - c * NS).astype(np.int64)
        cs.eew = np.asarray(ew[m], np.float32)
        d = np.bincount(cs.ecol, minlength=NS)
        cs.deg = d
        cs.Lv = np.maximum(4, ((d + 1 + 3) // 4) * 4)   # slots incl self
        cores.append(cs)

    Ls = sorted(set(np.unique(np.concatenate([cs.Lv for cs in cores])).tolist()))
    st.Ls = Ls
    nbj = {}
    for L in Ls:
        mx = max(int((cs.Lv == L).sum()) for cs in cores)
        nbj[L] = _ceil(max(mx, 1), P)
    st.nbj = nbj
    nbtot = sum(nbj.values())
    nbtot = _ceil(nbtot, 32) * 32          # CB = 128*nbtot divisible by 4096
    st.nbtot = nbtot
    st.CB = P * nbtot

    boff = {}
    woff = {}
    vo1 = {}
    vo2 = {}
    ob = ow = o1 = o2 = 0
    for L in Ls:
        boff[L] = ob
        woff[L] = ow
        vo1[L] = o1
        vo2[L] = o2
        ob += nbj[L]
        ow += nbj[L] * L
        o1 += nbj[L] * 3 * L
        o2 += nbj[L] * 7 * L
    st.boff, st.woff, st.vo1, st.vo2 = boff, woff, vo1, vo2
    st.SW, st.SV1, st.SV2 = ow, o1, o2

    woff_a = np.zeros(Ls[-1] + 1, np.int64)
    vo1_a = np.zeros(Ls[-1] + 1, np.int64)
    vo2_a = np.zeros(Ls[-1] + 1, np.int64)
    boff_a = np.zeros(Ls[-1] + 1, np.int64)
    nbj_a = np.zeros(Ls[-1] + 1, np.int64)
    for L in Ls:
        woff_a[L] = woff[L]
        vo1_a[L] = vo1[L]
        vo2_a[L] = vo2[L]
        boff_a[L] = boff[L]
        nbj_a[L] = nbj[L]

    for cs in cores:
        # node placement: sort nodes by (L, id); rank within class
        ordn = np.argsort(cs.Lv, kind='stable')
        Lsrt = cs.Lv[ordn]
        first = np.r_[True, Lsrt[1:] != Lsrt[:-1]]
        start = np.maximum.accumulate(np.where(first, np.arange(NS), 0))
        rank = np.arange(NS) - start
        nb_n = nbj_a[Lsrt]
        p_s = rank // nb_n
        bl_s = rank % nb_n
        cs.node_p = np.empty(NS, np.int64)
        cs.node_b = np.empty(NS, np.int64)          # global block idx
        cs.node_bl = np.empty(NS, np.int64)         # block idx within class
        cs.node_p[ordn] = p_s
        cs.node_bl[ordn] = bl_s
        cs.node_b[ordn] = boff_a[Lsrt] + bl_s
        # per-node flat bases (l-major layout: idx = l*F*nb + f*nb + b)
        cs.nnb = nbj_a[cs.Lv]
        cs.wbase = woff_a[cs.Lv] + cs.node_bl            # self slot l=0
        cs.v1base = vo1_a[cs.Lv] + cs.node_bl
        cs.v2base = vo2_a[cs.Lv] + cs.node_bl
        # edge slot index: 1 + occurrence among edges sharing this dest
        orde = np.argsort(cs.ecol, kind='stable')
        ecs = cs.ecol[orde]
        firste = np.r_[True, ecs[1:] != ecs[:-1]]
        starte = np.maximum.accumulate(np.where(firste, np.arange(len(ecs)), 0))
        k = np.empty(len(ecs), np.int64)
        k[orde] = np.arange(len(ecs)) - starte + 1
        cs.ek = k
        cs.ep = cs.node_p[cs.ecol]
        cs.enb = nbj_a[cs.Lv[cs.ecol]]
        ebl = cs.node_bl[cs.ecol]
        cs.e_w = woff_a[cs.Lv[cs.ecol]] + k * cs.enb + ebl
        cs.e_v1 = vo1_a[cs.Lv[cs.ecol]] + k * 3 * cs.enb + ebl
        cs.e_v2 = vo2_a[cs.Lv[cs.ecol]] + k * 7 * cs.enb + ebl
    st.cores = cores
    return st


# --------------------------------------------------------------------------
# device programs
def _slabs(total, n):
    out = []
    step = _ceil(total, n)
    o = 0
    while o < total:
        out.append((o, min(step, total - o)))
        o += step
    return out


def build_PA(st):
    """ews[P,SW]f32, x[P,nbtot*3]f32 -> ewp[P,SW]bf16, xp bf16, disb bf16."""
    nc = bass.Bass("TRN2", num_devices=S)
    SW, nbtot = st.SW, st.nbtot
    ews = nc.dram_tensor("ews", (P, SW), FP, kind="ExternalInput")
    x_in = nc.dram_tensor("x", (P, nbtot * 3), FP, kind="ExternalInput")
    ewp_o = nc.dram_tensor("ewp", (P, SW), BF, kind="ExternalOutput")
    xp_o = nc.dram_tensor("xp", (P, nbtot * 3), BF, kind="ExternalOutput")
    disb_o = nc.dram_tensor("disb", (P, nbtot), BF, kind="ExternalOutput")
    with tile.TileContext(nc) as tc:
        with tc.tile_pool(name="big", bufs=1) as bpool, tc.tile_pool(
            name="str", bufs=4
        ) as pool:
            t_ewb = bpool.tile([P, SW], BF)
            t_deg = bpool.tile([P, nbtot], FP)
            nc.vector.memset(t_deg[:], 1.0)
            qs = [nc.sync, nc.scalar]
            qi = 0
            order = sorted(st.Ls, key=lambda L: L * st.nbj[L])
            for L in order:
                nb = st.nbj[L]
                w0 = st.woff[L]
                t_e = pool.tile([P, L * nb], FP, tag="ein")
                qs[qi % 2].dma_start(
                    out=t_e[:], in_=ews[:, w0: w0 + L * nb]
                )
                qi += 1
                # bf16 copy (4x) for the later ew' multiply
                nc.vector.tensor_copy(
                    out=t_ewb[:, w0: w0 + L * nb], in_=t_e[:]
                )
                msgA = t_e[:].rearrange("p (l b) -> p l b", l=L)
                s = L
                while s > 2:
                    h = (s + 1) // 2
                    t = s - h
                    nc.vector.tensor_tensor(
                        msgA[:, 0:t], msgA[:, 0:t], msgA[:, h:s], ADD
                    )
                    s = h
                nc.vector.tensor_tensor(
                    t_deg[:, st.boff[L]: st.boff[L] + nb],
                    msgA[:, 0],
                    msgA[:, 1],
                    ADD,
                )
            # dis = rsqrt(deg): sqrt (Act) + reciprocal + one Newton step
            t_sq = bpool.tile([P, nbtot], FP)
            nc.scalar.sqrt(t_sq[:], t_deg[:])
            t_r = bpool.tile([P, nbtot], FP)
            nc.vector.reciprocal(t_r[:], t_sq[:])
            t_y2 = bpool.tile([P, nbtot], FP)
            nc.vector.tensor_tensor(t_y2[:], t_r[:], t_r[:], MUL)
            nc.vector.tensor_tensor(t_y2[:], t_y2[:], t_deg[:], MUL)
            nc.vector.tensor_scalar_mul(t_y2[:], t_y2[:], -0.5)
            nc.vector.tensor_scalar_add(t_y2[:], t_y2[:], 1.5)
            nc.vector.tensor_tensor(t_r[:], t_r[:], t_y2[:], MUL)
            # disb (bf16)
            t_db = bpool.tile([P, nbtot], BF)
            nc.vector.tensor_copy(out=t_db[:], in_=t_r[:])
            nc.sync.dma_start(out=disb_o[:], in_=t_db[:])
            # xp = x * dis (bf16)
            t_x = bpool.tile([P, nbtot * 3], FP)
            nc.scalar.dma_start(out=t_x[:], in_=x_in[:])
            t_xp = bpool.tile([P, nbtot * 3], BF)
            nc.vector.tensor_tensor(
                t_xp[:].rearrange("p (b f) -> p b f", f=3),
                t_x[:].rearrange("p (b f) -> p b f", f=3),
                t_r[:].unsqueeze(2).broadcast_to([P, nbtot, 3]),
                MUL,
            )
            nc.sync.dma_start(out=xp_o[:], in_=t_xp[:])
            # ewp = ewb * dis[col] (all-bf16 -> 2x), in-place on ewb
            for L in st.Ls:
                nb = st.nbj[L]
                w0 = st.woff[L]
                ew_v = t_ewb[:, w0: w0 + L * nb].rearrange(
                    "p (l b) -> p l b", l=L
                )
                nc.vector.tensor_tensor(
                    ew_v,
                    ew_v,
                    t_db[:, st.boff[L]: st.boff[L] + nb]
                    .unsqueeze(1)
                    .broadcast_to([P, L, nb]),
                    MUL,
                )
            for i, (o, ln) in enumerate(_slabs(SW, 4)):
                qs[i % 2].dma_start(
                    out=ewp_o[:, o: o + ln], in_=t_ewb[:, o: o + ln]
                )
    legalize_waits(nc)
    return nc


def _edge_stream(nc, st, F, pool, t_ewp, val_dram, vo, out_ap_fn,
                 sort_small=False, split_big=False):
    # NOTE: GpSimd shares SBUF ports with DVE — concurrent TTs on both
    # engines throttle each other badly (measured), so no Pool compute.
    """Per class: stream the [P, L, F, nb] value slab, multiply by the
    resident ew' slab (broadcast over F), fold the slot dim (outermost ->
    contiguous slabs), final add into out_ap_fn(L) ([P, F, nb] view)."""
    qs = [nc.sync, nc.scalar]
    qi = 0
    order = (sorted(st.Ls, key=lambda L: L * st.nbj[L])
             if sort_small else list(st.Ls))
    for L in order:
        nb = st.nbj[L]
        elems = L * F * nb
        t_in = pool.tile([P, elems], BF, tag="vin")
        if split_big and elems > 4096:
            # split the transfer along l (outer dim -> contiguous halves)
            h = (L // 2) * F * nb
            qs[qi % 2].dma_start(
                out=t_in[:, 0:h], in_=val_dram[:, vo[L]: vo[L] + h]
            )
            qs[(qi + 1) % 2].dma_start(
                out=t_in[:, h:elems],
                in_=val_dram[:, vo[L] + h: vo[L] + elems],
            )
            qi += 2
        else:
            qs[qi % 2].dma_start(
                out=t_in[:], in_=val_dram[:, vo[L]: vo[L] + elems]
            )
            qi += 1
        eng = nc.vector
        msg4 = t_in[:].rearrange("p (l f b) -> p l f b", l=L, f=F)
        eng.tensor_tensor(
            msg4,
            msg4,
            t_ewp[:, st.woff[L]: st.woff[L] + L * nb]
            .rearrange("p (l b) -> p l b", l=L)
            .unsqueeze(2)
            .broadcast_to([P, L, F, nb]),
            MUL,
        )
        msg2 = t_in[:].rearrange("p (l x) -> p l x", l=L)
        s = L
        while s > 2:
            h = (s + 1) // 2
            t = s - h
            eng.tensor_tensor(msg2[:, 0:t], msg2[:, 0:t], msg2[:, h:s], ADD)
            s = h
        eng.tensor_tensor(out_ap_fn(L), msg4[:, 0], msg4[:, 1], ADD)


def build_PB(st):
    """xv[P,SV1]bf16 + ewp + disb + weights -> ys2z[(28, CB//4)]bf16."""
    nc = bass.Bass("TRN2", num_devices=S)
    SW, SV1, nbtot, CB = st.SW, st.SV1, st.nbtot, st.CB
    NT = CB // 2048
    xv = nc.dram_tensor("xv", (P, SV1), BF, kind="ExternalInput")
    ewp = nc.dram_tensor("ewp", (P, SW), BF, kind="ExternalInput")
    disb = nc.dram_tensor("disb", (P, nbtot), BF, kind="ExternalInput")
    w1t = nc.dram_tensor("w1t", (P, 16), FP, kind="ExternalInput")
    w2bd = nc.dram_tensor("w2bd", (P, 28), FP, kind="ExternalInput")
    ys2z_o = nc.dram_tensor("ys2z", (28, CB // 4), BF, kind="ExternalOutput")
    with tile.TileContext(nc) as tc:
        with tc.tile_pool(name="big", bufs=1) as bpool, tc.tile_pool(
            name="str", bufs=3
        ) as pool, tc.tile_pool(
            name="zt", bufs=3
        ) as zpool, tc.tile_pool(
            name="ys", bufs=2
        ) as ypool, tc.tile_pool(
            name="zp", bufs=4, space="PSUM"
        ) as zppool, tc.tile_pool(
            name="yp", bufs=2, space="PSUM"
        ) as yppool:
            t_ewp = bpool.tile([P, SW], BF)
            s0 = min(1024, SW)
            nc.sync.dma_start(out=t_ewp[:, 0:s0], in_=ewp[:, 0:s0])
            for i, (o, ln) in enumerate(_slabs(SW - s0, 3)):
                nc.scalar.dma_start(
                    out=t_ewp[:, s0 + o: s0 + o + ln],
                    in_=ewp[:, s0 + o: s0 + o + ln],
                )
            t_db = bpool.tile([P, nbtot], BF)
            nc.sync.dma_start(out=t_db[:], in_=disb[:])
            t_w1 = bpool.tile([P, 16], FP)
            nc.sync.dma_start(out=t_w1[:], in_=w1t[:])
            t_w1e = bpool.tile([4, 32], BF)
            nc.vector.memset(t_w1e[:], 0.0)
            nc.vector.tensor_copy(out=t_w1e[:, 0:16], in_=t_w1[0:4, :])
            t_w2 = bpool.tile([P, 28], FP)
            nc.sync.dma_start(out=t_w2[:], in_=w2bd[:])
            t_w2b = bpool.tile([P, 28], BF)
            nc.vector.tensor_copy(out=t_w2b[:], in_=t_w2[:])

            # v32 planar [P, 4, nbtot] fp32 (4th row unused until vd)
            t_v32 = bpool.tile([P, 4 * nbtot], FP)
            nc.vector.memset(t_v32[:], 0.0)
            v32_v = t_v32[:].rearrange("p (f b) -> p f b", f=4)

            def v32_out(L):
                return v32_v[:, 0:3, st.boff[L]: st.boff[L] + st.nbj[L]]

            _edge_stream(nc, st, 3, pool, t_ewp, xv, st.vo1, v32_out)

            # vd[p, f, b] = v32[p, f, b] * dis[p, b];  vd[p, 3, b] = dis
            t_vd = bpool.tile([P, 4 * nbtot], BF)
            vd_v = t_vd[:].rearrange("p (f b) -> p f b", f=4)
            nc.vector.tensor_tensor(
                vd_v[:, 0:3],
                v32_v[:, 0:3],
                t_db[:].unsqueeze(1).broadcast_to([P, 3, nbtot]),
                MUL,
            )
            nc.vector.tensor_copy(
                out=t_vd[:, 3 * nbtot: 4 * nbtot], in_=t_db[:]
            )
            # flatten to moving tensor rhs [4, CB]
            t_rhs = bpool.tile([4, CB], BF)
            fq = [nc.scalar, nc.sync, nc.scalar, nc.sync]
            for f in range(4):
                fq[f].dma_start(
                    out=t_rhs[f: f + 1, :],
                    in_=t_vd[:, f * nbtot: (f + 1) * nbtot],
                )
            # MLP: per 2048-col tile: 4 stripe matmuls -> relu -> W2bd -> out
            for t in range(NT):
                t_zp = zppool.tile([P, 512], FP, space="PSUM")
                for i in range(4):
                    c0 = t * 2048 + i * 512
                    nc.tensor.matmul(
                        out=t_zp[32 * i: 32 * (i + 1), :],
                        lhsT=t_w1e[:],
                        rhs=t_rhs[:, c0: c0 + 512],
                        start=True,
                        stop=True,
                        tile_position=(0, 32 * i),
                    )
                t_z = zpool.tile([P, 512], BF, tag="z")
                nc.scalar.activation(
                    out=t_z[:],
                    in_=t_zp[:],
                    func=mybir.ActivationFunctionType.Relu,
                )
                t_yp = yppool.tile([28, 512], FP, space="PSUM")
                nc.tensor.matmul(
                    out=t_yp[:],
                    lhsT=t_w2b[:],
                    rhs=t_z[:],
                    start=True,
                    stop=True,
                    tile_position=(0, 0),
                )
                t_ys = ypool.tile([28, 512], BF, tag="yo")
                nc.vector.tensor_copy(out=t_ys[:], in_=t_yp[:])
                [nc.sync, nc.scalar][t % 2].dma_start(
                    out=ys2z_o[:, t * 512: (t + 1) * 512], in_=t_ys[:]
                )
    legalize_waits(nc)
    return nc


def build_PC(st):
    """ysv[P,SV2]bf16 + ewp + b2 -> out[P,nbtot*7]f32."""
    nc = bass.Bass("TRN2", num_devices=S)
    SW, SV2, nbtot = st.SW, st.SV2, st.nbtot
    ysv = nc.dram_tensor("ysv", (P, SV2), BF, kind="ExternalInput")
    ewp = nc.dram_tensor("ewp", (P, SW), BF, kind="ExternalInput")
    b2t = nc.dram_tensor("b2t", (P, 7), FP, kind="ExternalInput")
    out_o = nc.dram_tensor("out", (P, nbtot * 7), FP, kind="ExternalOutput")
    with tile.TileContext(nc) as tc:
        with tc.tile_pool(name="big", bufs=1) as bpool, tc.tile_pool(
            name="str", bufs=4
        ) as pool:
            t_ewp = bpool.tile([P, SW], BF)
            s0 = min(1024, SW)
            nc.sync.dma_start(out=t_ewp[:, 0:s0], in_=ewp[:, 0:s0])
            for i, (o, ln) in enumerate(_slabs(SW - s0, 3)):
                nc.scalar.dma_start(
                    out=t_ewp[:, s0 + o: s0 + o + ln],
                    in_=ewp[:, s0 + o: s0 + o + ln],
                )
            t_b2 = bpool.tile([P, 7], FP)
            nc.sync.dma_start(out=t_b2[:], in_=b2t[:])
            # v2 class-planar: per class a contiguous [P, 7, nb] slab at 7*boff
            t_v2 = bpool.tile([P, nbtot * 7], FP)

            def v2_out(L):
                nb = st.nbj[L]
                return t_v2[:, 7 * st.boff[L]: 7 * (st.boff[L] + nb)] \
                    .rearrange("p (f b) -> p f b", f=7)

            _edge_stream(nc, st, 7, pool, t_ewp, ysv, st.vo2, v2_out,
                         sort_small=True, split_big=True)
            # out = v2 + b2 (per class slab), store per class to overlap;
            # the bias op is elided when b2 is identically zero
            qs = [nc.sync, nc.scalar]
            for i, L in enumerate(st.Ls):
                nb = st.nbj[L]
                if not st.b2_zero:
                    v = v2_out(L)
                    nc.vector.tensor_tensor(
                        v,
                        v,
                        t_b2[:].unsqueeze(2).broadcast_to([P, 7, nb]),
                        ADD,
                    )
                o0 = 7 * st.boff[L]
                qs[i % 2].dma_start(
                    out=out_o[:, o0: o0 + 7 * nb],
                    in_=t_v2[:, o0: o0 + 7 * nb],
                )
    legalize_waits(nc)
    return nc


# --------------------------------------------------------------------------
def kernel(x, edge_index, edge_weight, W1, b1, W2, b2):
    x = np.asarray(x, np.float32)
    ei = np.asarray(edge_index)
    ew = np.asarray(edge_weight, np.float32)
    W1 = np.asarray(W1, np.float32)
    b1 = np.asarray(b1, np.float32)
    W2 = np.asarray(W2, np.float32)
    b2 = np.asarray(b2, np.float32)

    PROGRAM_TIMES_NS.clear()
    st = build_structs(ei[0], ei[1], ew)
    st.b2_zero = not np.any(b2)
    _LAST_ST[0] = st
    nbtot, SW, SV1, SV2, CB = st.nbtot, st.SW, st.SV1, st.SV2, st.CB

    # ---------------- P_A ----------------
    nc = build_PA(st)
    in_maps = []
    for c in range(S):
        cs = st.cores[c]
        ews = np.zeros((P, SW), np.float32)
        # all self slots (incl pad-in-class nodes) -> 1.0 so deg>=1, no NaN
        for L in st.Ls:
            ews[:, st.woff[L]: st.woff[L] + st.nbj[L]] = 1.0   # l=0 plane
        ews[cs.ep, cs.e_w] = cs.eew             # edge slots (k>=1)
        x_t = np.zeros((P, nbtot, 3), np.float32)
        x_t[cs.node_p, cs.node_b] = x[c * NS: (c + 1) * NS]
        in_maps.append({"ews": ews, "x": x_t.reshape(P, nbtot * 3)})
    res = _run(nc, in_maps, "PA_prep")
    ewp_s = [np.asarray(res[c]["ewp"], bfloat16) for c in range(S)]
    xp_s = [np.asarray(res[c]["xp"], bfloat16).reshape(P, nbtot, 3)
            for c in range(S)]
    disb_s = [np.asarray(res[c]["disb"], bfloat16) for c in range(S)]

    # canonical per-global-node xp (pure byte movement)
    xp_can = np.zeros((N, 3), bfloat16)
    for c in range(S):
        cs = st.cores[c]
        xp_can[c * NS: (c + 1) * NS] = xp_s[c][cs.node_p, cs.node_b]

    # ---------------- P_B ----------------
    nc = build_PB(st)
    w1t = np.zeros((P, 16), np.float32)
    w1t[0:3] = W1
    w1t[3] = b1
    w2bd = np.zeros((P, 28), np.float32)
    for g in range(4):
        w2bd[g * 32: g * 32 + 16, g * 7: g * 7 + 7] = W2
    in_maps = []
    for c in range(S):
        cs = st.cores[c]
        xv = np.zeros((P, SV1), bfloat16)
        xrows = xp_can[cs.erow]                     # [E, 3] bf16
        for f in range(3):
            xv[cs.ep, cs.e_v1 + f * cs.enb] = xrows[:, f]
        xself = xp_can[c * NS: (c + 1) * NS]
        for f in range(3):
            xv[cs.node_p, cs.v1base + f * cs.nnb] = xself[:, f]
        in_maps.append(
            {"xv": xv, "ewp": ewp_s[c], "disb": disb_s[c],
             "w1t": w1t, "w2bd": w2bd}
        )
    res = _run(nc, in_maps, "PB_l1_mlp")
    ys2z_s = [np.asarray(res[c]["ys2z"], bfloat16) for c in range(S)]

    # unpermute ys2: node flat=(p*nbtot+b) -> chunk=flat//512, stripe=chunk%4,
    # tile=chunk//4, ys2[n,o] = ys2z[7*stripe+o, tile*512 + flat%512]
    ys2_can = np.zeros((N, 7), bfloat16)
    for c in range(S):
        cs = st.cores[c]
        flat = cs.node_p * nbtot + cs.node_b
        chunk = flat // 512
        cc = flat % 512
        rowi = 7 * (chunk % 4)
        coli = (chunk // 4) * 512 + cc
        for o in range(7):
            ys2_can[c * NS: (c + 1) * NS, o] = ys2z_s[c][rowi + o, coli]

    # ---------------- P_C ----------------
    nc = build_PC(st)
    b2t = np.broadcast_to(b2.reshape(1, 7), (P, 7)).copy()
    in_maps = []
    for c in range(S):
        cs = st.cores[c]
        ysv = np.zeros((P, SV2), bfloat16)
        yrows = ys2_can[cs.erow]
        for f in range(7):
            ysv[cs.ep, cs.e_v2 + f * cs.enb] = yrows[:, f]
        yself = ys2_can[c * NS: (c + 1) * NS]
        for f in range(7):
            ysv[cs.node_p, cs.v2base + f * cs.nnb] = yself[:, f]
        in_maps.append({"ysv": ysv, "ewp": ewp_s[c], "b2t": b2t})
    res = _run(nc, in_maps, "PC_l2")

    # out is class-planar: node value f at [p, 7*boff + f*nb + bl]
    boff_a = np.zeros(st.Ls[-1] + 1, np.int64)
    for L in st.Ls:
        boff_a[L] = st.boff[L]
    out = np.zeros((N, 7), np.float32)
    for c in range(S):
        cs = st.cores[c]
        o = np.asarray(res[c]["out"], np.float32)
        base = 7 * boff_a[cs.Lv] + cs.node_bl
        for f in range(7):
            out[c * NS: (c + 1) * NS, f] = o[cs.node_p, base + f * cs.nnb]
    return out
